# revision 1
# baseline (speedup 1.0000x reference)
"""TRN2 Bass kernel for nn_DSSMEmbed (vq_codebook).

Strategy (8 NeuronCores, data-parallel over batch, 256 imgs/core):
  - Activation layout: partitions = (x, channel) rows, free = (y, img).
  - 3x3 convs as Toeplitz matmuls over x-windows with batch streamed in N;
    dy handled by PSUM accumulation at shifted free-dim (y) offsets.
  - emb conv: 64x32 8-tile mode, windowed one-hot input from DRAM (K=56).
  - c1 conv:  128x32 4-col-tile mode, K=128 direct from a duplicated
    3-chunk natural layout (x0..7 / x4..11 / x8..15) -- no window copies.
  - c2 conv:  64x32 8-tile mode, windowed y-pair buffers built by DMA.
  - Embedding lookup folded into emb conv: host ships windowed one-hot(s)
    (bf16, tower1) and one-hot(s')-one-hot(s) (fp32 delta, tower2); the
    renormed embedding table is folded into the conv operator on host.
  - Tower2 (feeds VQ argmax) entirely fp32; tower1 + final BxB matmul bf16.
  - VQ: scores.T via PE (fp32), per-row max/max_index on DVE, indirect-DMA
    gather of zn rows, PE transpose, AllGather, local bf16 (256,512)@(512,2048).
  - embed1 norms via ones-matmul of squares; 1/(|e|+eps) and exp(scale)
    folded into the final evacuation as per-partition scalars.
"""
import sys

sys.path.insert(0, "/opt/trn_rl_repo")

import numpy as np
import concourse.bass as bass
import concourse.bacc as bacc
import concourse.mybir as mybir
import concourse.tile as tile
from concourse.bass_utils import run_bass_kernel_spmd

F32 = mybir.dt.float32
BF16 = mybir.dt.bfloat16
U32 = mybir.dt.uint32
AF = mybir.ActivationFunctionType

NCORES = 8
B = 2048
BL = B // NCORES          # 256 imgs per core
H = W = 16
DICT, SE, CE, ESZ, NZ = 14, 8, 16, 512, 512
EPS = 1e-4
YB = H * BL               # free dim (y, img) = 4096

DEBUG = False

# ---------------------------------------------------------------------------
# host-side preprocessing
# ---------------------------------------------------------------------------


def make_windowed_oh(nat):
    """nat: (DICT, H, W, Bloc) one-hot -> (4, 4, 128, 6, Bloc).

    px=2: 8 blocks; tensor t holds block t at rows 0.. and block t+4 at
    rows 64..; rows w*14+d for window x' = 2b-1+w, w in 0..3.  Second dim
    is the y-quarter: quarter q covers global y in [4q-1, 4q+5) (clipped,
    duplicated halo) so each DMA load is contiguous per partition.
    """
    out = np.zeros((4, 4, 128, 6, nat.shape[-1]), dtype=np.int8)
    for b in range(8):
        t, h = b % 4, b // 4
        for w in range(4):
            xs = 2 * b - 1 + w
            if 0 <= xs < W:
                for q in range(4):
                    ys, ye = max(0, 4 * q - 1), min(H, 4 * q + 5)
                    out[t, q, h * 64 + w * DICT:h * 64 + (w + 1) * DICT,
                        ys - (4 * q - 1):ye - (4 * q - 1)] = nat[:, ys:ye, xs, :]
    return out


def op_emb_win(wfold):
    """Folded emb conv operator for 64x32 windowed scheme: (3, 4, 128, 32).

    wfold: (C_out=16, DICT, 3, 3).  lhsT[dy, t, h*64 + w*14 + d,
    xr*16 + co] = wfold[co, d, dy, w - xr] (dx = w - xr in 0..2).
    """
    op = np.zeros((3, 4, 128, 32), dtype=np.float32)
    for dy in range(3):
        blk = np.zeros((56, 32), dtype=np.float32)
        for w in range(4):
            for xr in range(2):
                dx = w - xr
                if 0 <= dx <= 2:
                    blk[w * DICT:(w + 1) * DICT, xr * 16:(xr + 1) * 16] = \
                        wfold[:, :, dy, dx].T
        for h in range(2):
            op[dy, :, h * 64:h * 64 + 56, :] = blk[None]
    return op


def op_conv_win(wc, c_in, c_out):
    """Windowed 64-row conv operator: (3, 4, 128, px*c_out) with px=2.

    wc: (c_out, c_in, 3, 3).  Tensor t serves blocks b=t (rows 0..) and
    b=t+4 (rows 64..); rows w*c_in+ci for window x' = 2b-1+w (w in 0..3),
    cols xr*c_out+co.  Boundary rows (x'=-1 for b=0, x'=16 for b=7) are
    zeroed (matching window zero padding).
    """
    M = 2 * c_out
    op = np.zeros((3, 4, 128, M), dtype=np.float32)
    blk = np.zeros((4 * c_in, M), dtype=np.float32)
    for dy in range(3):
        blk[:] = 0.0
        for w in range(4):
            for xr in range(2):
                dx = w - xr
                if 0 <= dx <= 2:
                    blk[w * c_in:(w + 1) * c_in, xr * c_out:(xr + 1) * c_out] = \
                        wc[:, :, dy, dx].T
        for h in range(2):
            op[dy, :, h * 64:h * 64 + 4 * c_in, :] = blk[None]
        op[dy, 0, 0:c_in, :] = 0.0                    # b=0, w=0 (x'=-1)
        op[dy, 3, 64 + 3 * c_in:64 + 4 * c_in, :] = 0.0  # b=7, w=3 (x'=16)
    return op


def host_prep(inputs):
    s = np.asarray(inputs["s"])
    sp = np.asarray(inputs["s_prime"])
    se_w = np.asarray(inputs["state_embed"], dtype=np.float32)
    norms = np.sqrt((se_w * se_w).sum(1, keepdims=True))
    table = se_w / np.maximum(norms, 1.0)

    oh_s = (np.arange(DICT)[:, None, None, None] ==
            s.transpose(1, 2, 0)[None]).astype(np.float32)
    oh_sp = (np.arange(DICT)[:, None, None, None] ==
             sp.transpose(1, 2, 0)[None]).astype(np.float32)
    oh_d = oh_sp - oh_s

    emb_fold = np.einsum("oikl,di->odkl",
                         np.asarray(inputs["conv_embed_w"], np.float32), table)

    shared = {
        "op_emb": op_emb_win(emb_fold),
        "op_c1t1": op_conv_win(np.asarray(inputs["p1c1_w"], np.float32), 16, 16),
        "op_c1t2": op_conv_win(np.asarray(inputs["p2c1_w"], np.float32), 16, 16),
        "op_c2t1": op_conv_win(np.asarray(inputs["p1c2_w"], np.float32), 16, 32),
        "op_c2t2": op_conv_win(np.asarray(inputs["p2c2_w"], np.float32), 16, 32),
    }

    def reorder_lin(lw):
        # K order: (chunk c, y, row r), r = xr*32+ch, x = c*4+xr
        lw = np.asarray(lw, np.float32).reshape(ESZ, 32, H, W)
        lw = lw.transpose(3, 1, 2, 0).reshape(4, 4, 32, H, ESZ)  # (c,xr,ch,y,E)
        return np.ascontiguousarray(
            lw.transpose(0, 3, 1, 2, 4).reshape(4, H, 128, ESZ).reshape(64, 128, ESZ))

    shared["lw_t1"] = reorder_lin(inputs["p1l_w"])
    shared["lw_t2"] = reorder_lin(inputs["p2l_w"])

    zv = np.asarray(inputs["z_vectors"], np.float32)
    zn = zv / np.sqrt((zv * zv).sum(1, keepdims=True))
    shared["zn"] = zn
    shared["znT"] = np.ascontiguousarray(zn.T)

    def conv_bias(bvec, c_out):
        reps = 128 // c_out
        return np.ascontiguousarray(
            np.tile(np.asarray(bvec, np.float32), reps)[:, None])

    shared["b_emb"] = conv_bias(inputs["conv_embed_b"], 16)
    shared["b_c1t1"] = conv_bias(inputs["p1c1_b"], 16)
    shared["b_c1t2"] = conv_bias(inputs["p2c1_b"], 16)
    shared["b_c2t1"] = conv_bias(inputs["p1c2_b"], 32)
    shared["b_c2t2"] = conv_bias(inputs["p2c2_b"], 32)
    shared["b_l1"] = np.ascontiguousarray(
        np.asarray(inputs["p1l_b"], np.float32).reshape(1, ESZ))
    shared["b_l2"] = np.ascontiguousarray(
        np.asarray(inputs["p2l_b"], np.float32).reshape(1, ESZ))

    esc = float(np.exp(np.asarray(inputs["scale"], np.float32).reshape(-1)[0]))

    percore = []
    for c in range(NCORES):
        sl = slice(c * BL, (c + 1) * BL)
        percore.append({
            "ohs": make_windowed_oh(oh_s[..., sl]),
            "ohd": make_windowed_oh(oh_d[..., sl]),
        })
    return shared, percore, esc


# ---------------------------------------------------------------------------
# device program
# ---------------------------------------------------------------------------


def _clip_dy(y0, ny, dy):
    s = max(y0, -dy)
    e = min(y0 + ny, H - dy)
    if s >= e:
        return None
    return (s - y0) * BL, (e - s) * BL, s + dy


def build_program(esc, debug=False):
    from contextlib import ExitStack
    nc = bacc.Bacc("TRN2", target_bir_lowering=False, debug=False,
                   num_devices=NCORES)

    def din(name, shape, dt):
        return nc.dram_tensor(name, list(shape), dt, kind="ExternalInput").ap()

    ohs_d = din("ohs", (4, 4, 128, 6, BL), mybir.dt.int8)
    ohd_d = din("ohd", (4, 4, 128, 6, BL), mybir.dt.int8)
    op_embt1_d = din("op_embt1", (3, 4, 128, 32), BF16)
    op_embt2_d = din("op_embt2", (3, 4, 128, 32), F32)
    op_c1t1_d = din("op_c1t1", (3, 4, 128, 32), BF16)
    op_c1t2_d = din("op_c1t2", (3, 4, 128, 32), F32)
    op_c2t1_d = din("op_c2t1", (3, 4, 128, 64), BF16)
    op_c2t2_d = din("op_c2t2", (3, 4, 128, 64), F32)
    lw1_d = din("lw1", (64, 128, ESZ), BF16)
    lw2_d = din("lw2", (64, 128, ESZ), F32)
    b_se_d = din("b_se", (128, 1), F32)
    b_c1t1_d = din("b_c1t1", (128, 1), F32)
    b_c1t2_d = din("b_c1t2", (128, 1), F32)
    b_c2t1_d = din("b_c2t1", (128, 1), F32)
    b_c2t2_d = din("b_c2t2", (128, 1), F32)
    b_l1_d = din("b_l1", (1, ESZ), F32)
    b_l2_d = din("b_l2", (1, ESZ), F32)
    znt_d = din("znt", (ESZ, NZ), F32)
    zn_d = din("zn", (NZ, ESZ), F32)
    ident_d = din("ident", (128, 128), F32)

    out_d = nc.dram_tensor("out", [BL, B], F32, kind="ExternalOutput").ap()
    dbg = {}
    if debug:
        for nm, shp, dt in [("dbg_e1", (2, 128, ESZ), F32),
                            ("dbg_e2", (4, 128, BL), F32),
                            ("dbg_sc", (2, 128, NZ), F32),
                            ("dbg_idx", (2, 128, 8), U32),
                            ("dbg_d", (2, 128, YB), F32),
                            ("dbg_se", (2, 128, YB), BF16),
                            ("dbg_c1a", (2, 128, YB), BF16),
                            ("dbg_c2a", (4, 128, YB), BF16),
                            ("dbg_c1", (2, 128, YB), F32),
                            ("dbg_c2", (4, 128, YB), F32)]:
            dbg[nm] = nc.dram_tensor(nm, list(shp), dt,
                                     kind="ExternalOutput").ap()

    zloc_d = nc.dram_tensor("zloc", [ESZ, BL], BF16).ap()
    zg_d = nc.dram_tensor("zg", [NCORES * ESZ, BL], BF16,
                          addr_space="Shared").ap()

    with tile.TileContext(nc) as tc, ExitStack() as ES:
        cst = ES.enter_context(tc.tile_pool(name="cst", bufs=1))
        npool = ES.enter_context(tc.tile_pool(name="nat", bufs=1))
        epool = ES.enter_context(tc.tile_pool(name="emb", bufs=1))

        ident_sb = cst.tile([128, 128], F32, tag="ident", name="ident")
        nc.sync.dma_start(ident_sb[:], ident_d[:])
        ones_sb = cst.tile([128, 1], F32, tag="ones", name="ones")
        nc.vector.memset(ones_sb[:], 1.0)
        bias_sb = {}
        for nm, d in [("b_se", b_se_d), ("b_c1t1", b_c1t1_d),
                      ("b_c1t2", b_c1t2_d), ("b_c2t1", b_c2t1_d),
                      ("b_c2t2", b_c2t2_d)]:
            t = cst.tile([128, 1], F32, tag=nm, name=nm)
            nc.sync.dma_start(t[:], d[:])
            bias_sb[nm] = t
        bl_sb = {}
        for nm, d in [("b_l1", b_l1_d), ("b_l2", b_l2_d)]:
            t = cst.tile([1, ESZ], F32, tag=f"{nm}r", name=f"{nm}r")
            nc.sync.dma_start(t[:], d[:])
            bl_sb[nm] = t
        ones_k = cst.tile([1, 128], F32, tag="ones_k", name="ones_k")
        nc.vector.memset(ones_k[:], 1.0)

        def load_ops(op_d, dt, width, nt, pfx):
            ops = [[cst.tile([128, width], dt, tag=f"{pfx}{dy}{t}",
                             name=f"{pfx}{dy}{t}") for t in range(nt)]
                   for dy in range(3)]
            for dy in range(3):
                for t in range(nt):
                    nc.sync.dma_start(ops[dy][t][:], op_d[dy, t])
            return ops

        ops_embt2 = load_ops(op_embt2_d, F32, 32, 4, "oe2")
        ops_embt1 = load_ops(op_embt1_d, BF16, 32, 4, "oe1")
        ops_c1t2 = load_ops(op_c1t2_d, F32, 32, 4, "oc12")
        ops_c1t1 = load_ops(op_c1t1_d, BF16, 32, 4, "oc11")
        ops_c2t2 = load_ops(op_c2t2_d, F32, 64, 4, "od12")
        ops_c2t1 = load_ops(op_c2t1_d, BF16, 64, 4, "od11")

        # ---------------- emb conv (64x32 8-tile, windowed DRAM input) ----
        def emb_conv(oh_d, ops, dt, bias, tags, wbufs=2):
            outs = [npool.tile([128, YB], dt, tag=tg, name=tg) for tg in tags]
            with tc.tile_pool(name=f"ew{tags[0]}", bufs=wbufs) as wp, \
                 tc.tile_pool(name=f"ep{tags[0]}", bufs=2, space="PSUM") as pp:
                for q in range(4):
                    wins = []
                    for t in range(4):
                        w = wp.tile([128, 6, BL], dt, tag=f"w{t}", name=f"w{t}")
                        nc.gpsimd.dma_start(w[:], oh_d[t, q])
                        wins.append(w)
                    for yg in (2 * q, 2 * q + 1):
                        y0 = 2 * yg
                        ps = [pp.tile([128, 2 * BL], F32, tag=f"p{i}", name=f"p{i}")
                              for i in range(2)]
                        first = True
                        for dy in (0, -1, 1):
                            n0, N, ysrc = _clip_dy(y0, 2, dy)
                            ly = ysrc - (4 * q - 1)
                            nys = N // BL
                            for b in range(8):
                                t, hh = b % 4, b // 4
                                nc.tensor.matmul(
                                    ps[hh][32 * (b % 4):32 * (b % 4) + 32,
                                           n0:n0 + N],
                                    ops[dy + 1][t][hh * 64:hh * 64 + 56, :],
                                    wins[t][hh * 64:hh * 64 + 56,
                                            ly:ly + nys, :],
                                    start=first, stop=(dy == 1),
                                    tile_position=(hh * 64, 32 * (b % 4)))
                            first = False
                        sl = slice(y0 * BL, (y0 + 2) * BL)
                        bb0 = bias[:] if bias is not None else 0.0
                        nc.scalar.activation(outs[0][:, sl], ps[0][:],
                                             AF.Identity, bias=bb0)
                        nc.scalar.activation(outs[1][:, sl], ps[1][:],
                                             AF.Identity, bias=bb0)
            return outs

        # -------- windowed x-pair builder: 2-chunk nat -> 4 win tensors ----
        def build_wins(nat2, dt, q, wp):
            """Window tensor t rows [h*64 + (x'-(2b-1))*16 + ci] with
            b = t + 4h, covering y-quarter q (global y in [4q-1, 4q+5))."""
            ys, ye = max(0, 4 * q - 1), min(H, 4 * q + 5)
            ly0, ly1 = ys - (4 * q - 1), ye - (4 * q - 1)
            wins = []
            for t in range(4):
                w = wp.tile([128, 6, BL], dt, tag=f"w{t}", name=f"w{t}")
                for hh in range(2):
                    b = t + 4 * hh
                    x0 = 2 * b - 1
                    if b == 0:
                        nc.vector.memset(w[0:64, :, :], 0.0)
                    if b == 7:
                        nc.vector.memset(w[64:128, :, :], 0.0)
                    xs_s, xs_e = max(0, x0), min(W, x0 + 4)
                    pieces = []
                    if xs_s < 8 < xs_e:
                        pieces = [(xs_s, 8), (8, xs_e)]
                    else:
                        pieces = [(xs_s, xs_e)]
                    for (a, bb) in pieces:
                        ch = a // 8
                        nc.sync.dma_start(
                            w[hh * 64 + (a - x0) * 16:hh * 64 + (bb - x0) * 16,
                              ly0:ly1, :],
                            nat2[ch].rearrange("p (y i) -> p y i", y=H)
                            [(a % 8) * 16:(a % 8) * 16 + (bb - a) * 16, ys:ye, :])
                wins.append(w)
            return wins

        # ---------------- c1 conv (64x32 8-tile, windowed) -----------------
        def c1_conv(ins2, ops, dt, bias, tags):
            outs = [npool.tile([128, YB], dt, tag=tg, name=tg) for tg in tags]
            with tc.tile_pool(name=f"cw{tags[0]}", bufs=2) as wp, \
                 tc.tile_pool(name=f"cp{tags[0]}", bufs=2, space="PSUM") as pp:
                for q in range(4):
                    wins = build_wins(ins2, dt, q, wp)
                    for yg in (2 * q, 2 * q + 1):
                        y0 = 2 * yg
                        ps = [pp.tile([128, 2 * BL], F32, tag=f"p{i}", name=f"p{i}")
                              for i in range(2)]
                        first = True
                        for dy in (0, -1, 1):
                            n0, N, ysrc = _clip_dy(y0, 2, dy)
                            ly = ysrc - (4 * q - 1)
                            nys = N // BL
                            for b in range(8):
                                t, hh = b % 4, b // 4
                                nc.tensor.matmul(
                                    ps[hh][32 * (b % 4):32 * (b % 4) + 32,
                                           n0:n0 + N],
                                    ops[dy + 1][t][hh * 64:hh * 64 + 64, :],
                                    wins[t][hh * 64:hh * 64 + 64, ly:ly + nys, :],
                                    start=first, stop=(dy == 1),
                                    tile_position=(hh * 64, 32 * (b % 4)))
                            first = False
                        sl = slice(y0 * BL, (y0 + 2) * BL)
                        for i in range(2):
                            nc.scalar.activation(outs[i][:, sl], ps[i][:],
                                                 AF.Relu, bias=bias[:])
            return outs

        # ---------------- c2 conv (64x64 4-tile, windowed) -----------------
        def c2_conv(ins2, ops, dt, bias, tags):
            outs = [npool.tile([128, YB], dt, tag=tg, name=tg) for tg in tags]
            BORD = [0, 1, 4, 5, 2, 3, 6, 7]
            with tc.tile_pool(name=f"dw{tags[0]}", bufs=2) as wp, \
                 tc.tile_pool(name=f"dp{tags[0]}", bufs=2, space="PSUM") as pp:
                for q in range(4):
                    wins = build_wins(ins2, dt, q, wp)
                    for yg in (2 * q, 2 * q + 1):
                        y0 = 2 * yg
                        ps = [pp.tile([128, 2 * BL], F32, tag=f"p{i}", name=f"p{i}")
                              for i in range(4)]
                        first = True
                        for dy in (0, -1, 1):
                            n0, N, ysrc = _clip_dy(y0, 2, dy)
                            ly = ysrc - (4 * q - 1)
                            nys = N // BL
                            for b in BORD:
                                t, hh = b % 4, b // 4
                                nc.tensor.matmul(
                                    ps[b // 2][64 * (b % 2):64 * (b % 2) + 64,
                                               n0:n0 + N],
                                    ops[dy + 1][t][hh * 64:hh * 64 + 64, :],
                                    wins[t][hh * 64:hh * 64 + 64, ly:ly + nys, :],
                                    start=first, stop=(dy == 1),
                                    tile_position=(hh * 64, 64 * (b % 2)))
                            first = False
                        sl = slice(y0 * BL, (y0 + 2) * BL)
                        for i in range(4):
                            nc.scalar.activation(outs[i][:, sl], ps[i][:],
                                                 AF.Relu, bias=bias[:])
            return outs

        # ---------------- linear (M=img, N=E; returns (img, E) chunks) ----
        def linear(c2o, lw_d, dt, bias_row, tagp, dma_eng=None):
            embT = [epool.tile([128, ESZ], F32, tag=f"{tagp}T{m}", name=f"{tagp}T{m}")
                    for m in range(2)]
            with tc.tile_pool(name=f"lw{tagp}", bufs=6) as lwp, \
                 tc.tile_pool(name=f"lp{tagp}", bufs=1, space="PSUM") as pp:
                ps = [pp.tile([128, ESZ], F32, tag=f"p{m}", name=f"p{m}")
                      for m in range(2)]
                for k in range(64):
                    cch, y = k // 16, k % 16
                    lwt = lwp.tile([128, ESZ], dt, tag="lw", name="lw")
                    (dma_eng or nc.scalar).dma_start(lwt[:], lw_d[k])
                    for m in range(2):
                        lhsT = c2o[cch][:, y * BL + 128 * m:y * BL + 128 * m + 128]
                        nc.tensor.matmul(ps[m][:], lhsT, lwt[:],
                                         start=(k == 0), stop=False)
                for m in range(2):
                    nc.tensor.matmul(ps[m][:], ones_k[:],
                                     bias_row[:], start=False, stop=True)
                for m in range(2):
                    nc.scalar.activation(embT[m][:], ps[m][:], AF.Identity)
            return embT

        def transpose_back(embT, dt, tagp):
            """(img,E) 2 chunks -> (E,img) 4 chunks of dtype dt."""
            emb = [epool.tile([128, BL], dt, tag=f"{tagp}{e}", name=f"{tagp}{e}")
                   for e in range(4)]
            with tc.tile_pool(name=f"tp{tagp}", bufs=2, space="PSUM") as tpp:
                for m in range(2):
                    for e in range(4):
                        tp = tpp.tile([128, 128], F32, tag="tp", name="tp")
                        nc.tensor.transpose(tp[:], embT[m][:, 128 * e:128 * e + 128],
                                            ident_sb[:])
                        nc.vector.tensor_copy(emb[e][:, 128 * m:128 * m + 128], tp[:])
            return emb

        # ================== tower 2 (fp32 delta path) ==================
        with nc.named_scope("t2emb"):
            d3 = emb_conv(ohd_d, ops_embt2, F32, None, ["A0", "A1"])
        if debug:
            for c in range(2):
                nc.sync.dma_start(dbg["dbg_d"][c], d3[c][:])
        with nc.named_scope("t2c1"):
            c1o2 = c1_conv(d3, ops_c1t2, F32, bias_sb["b_c1t2"],
                           ["B0", "B1"])
        if debug:
            for c in range(2):
                nc.sync.dma_start(dbg["dbg_c1"][c], c1o2[c][:])
        with nc.named_scope("t2c2"):
            c2o2 = c2_conv(c1o2, ops_c2t2, F32, bias_sb["b_c2t2"],
                           ["C0", "C1", "A0", "A1"])
        if debug:
            for c in range(4):
                nc.sync.dma_start(dbg["dbg_c2"][c], c2o2[c][:])
        with nc.named_scope("t2lin"):
            embT2 = linear(c2o2, lw2_d, F32, bl_sb["b_l2"], "e2")
            embed2 = transpose_back(embT2, F32, "e2n")
        if debug:
            for m in range(4):
                nc.sync.dma_start(dbg["dbg_e2"][m], embed2[m][:])

        # ================== VQ ==================
        # ================== tower 1 (bf16) ==================
        with nc.named_scope("t1emb"):
            se3 = emb_conv(ohs_d, ops_embt1, BF16, bias_sb["b_se"],
                           ["B0", "B1"])
        with nc.named_scope("t1c1"):
            c1o1 = c1_conv(se3, ops_c1t1, BF16, bias_sb["b_c1t1"],
                           ["C0", "C1"])
        ES.enter_context(nc.named_scope("vq"))
        with tc.tile_pool(name="vq", bufs=1) as vqp, \
             tc.tile_pool(name="vqp", bufs=1, space="PSUM") as vpp:
            pass_pool = None
            znt_sb = []
            for e in range(4):
                t = vqp.tile([128, NZ], F32, tag=f"znt{e}", name=f"znt{e}")
                nc.sync.dma_start(t[:], znt_d[128 * e:128 * e + 128, :])
                znt_sb.append(t)
            sps = [vpp.tile([128, NZ], F32, tag=f"s{m}", name=f"s{m}") for m in range(2)]
            for e in range(4):
                for m in range(2):
                    nc.tensor.matmul(sps[m][:],
                                     embed2[e][:, 128 * m:128 * m + 128],
                                     znt_sb[e][:], start=(e == 0),
                                     stop=(e == 3))
            idxs = []
            for m in range(2):
                sc = vqp.tile([128, NZ], F32, tag=f"sc{m}", name=f"sc{m}")
                nc.vector.tensor_copy(sc[:], sps[m][:])
                mx = vqp.tile([128, 8], F32, tag=f"mx{m}", name=f"mx{m}")
                nc.vector.max(mx[:], sc[:])
                ix = vqp.tile([128, 8], U32, tag=f"ix{m}", name=f"ix{m}")
                nc.vector.max_index(ix[:], mx[:], sc[:])
                idxs.append(ix)
                if debug:
                    nc.sync.dma_start(dbg["dbg_sc"][m], sc[:])
                    nc.sync.dma_start(dbg["dbg_idx"][m], ix[:])
            zt = [vqp.tile([128, BL], BF16, tag=f"zt{e}", name=f"zt{e}") for e in range(4)]
            for m in range(2):
                zl = vqp.tile([128, ESZ], F32, tag=f"zl{m}", name=f"zl{m}")
                nc.gpsimd.indirect_dma_start(
                    out=zl[:], out_offset=None, in_=zn_d[:],
                    in_offset=bass.IndirectOffsetOnAxis(ap=idxs[m][:, :1],
                                                        axis=0))
                for e in range(4):
                    tp = vpp.tile([128, 128], F32, tag="tp", name="tp")
                    nc.tensor.transpose(tp[:], zl[:, 128 * e:128 * e + 128],
                                        ident_sb[:])
                    nc.vector.tensor_copy(zt[e][:, 128 * m:128 * m + 128],
                                          tp[:])
            for e in range(4):
                nc.gpsimd.dma_start(zloc_d[128 * e:128 * e + 128, :], zt[e][:])
            nc.gpsimd.collective_compute(
                "AllGather", mybir.AluOpType.bypass,
                replica_groups=[list(range(NCORES))],
                ins=[zloc_d[:]], outs=[zg_d[:]])

        with nc.named_scope("t1c2"):
            c2o1 = c2_conv(c1o1, ops_c2t1, BF16, bias_sb["b_c2t1"],
                           ["A0", "A1", "B0", "B1"])
        with nc.named_scope("t1lin"):
            embT1 = linear(c2o1, lw1_d, BF16, bl_sb["b_l1"], "e1")
            e1b = transpose_back(embT1, BF16, "e1b")
        if debug:
            for c in range(2):
                nc.sync.dma_start(dbg["dbg_se"][c], se3[c][:])
            for c in range(2):
                nc.sync.dma_start(dbg["dbg_c1a"][c], c1o1[c][:])
            for c in range(4):
                nc.sync.dma_start(dbg["dbg_c2a"][c], c2o1[c][:])

        with tc.tile_pool(name="nrm", bufs=1) as nrp:
            rnt = [epool.tile([128, 1], F32, tag=f"rnt{m}", name=f"rnt{m}")
                   for m in range(2)]
            for m in range(2):
                sq = nrp.tile([128, ESZ], F32, tag="sq", name="sq")
                nc.vector.tensor_mul(sq[:], embT1[m][:], embT1[m][:])
                n2 = nrp.tile([128, 1], F32, tag="n2", name="n2")
                nc.vector.tensor_reduce(n2[:], sq[:], mybir.AxisListType.X,
                                        mybir.AluOpType.add)
                nc.scalar.sqrt(n2[:], n2[:])
                nc.vector.tensor_scalar_add(n2[:], n2[:], EPS)
                nc.vector.reciprocal(n2[:], n2[:])
                nc.vector.tensor_scalar_mul(rnt[m][:], n2[:], esc)

        # ================== final (bf16) ==================
        zgr = zg_d.rearrange("(c e p) i -> e p c i", c=NCORES, e=4, p=128)
        gsb = []
        for e in range(4):
            g = epool.tile([128, B], BF16, tag=f"g{e}", name=f"g{e}")
            nc.sync.dma_start(g[:], zgr[e])
            gsb.append(g)
        with tc.tile_pool(name="fin", bufs=1) as fqp, \
             tc.tile_pool(name="finp", bufs=2, space="PSUM") as fpp:
            osb = [fqp.tile([128, B], F32, tag=f"o{m}", name=f"o{m}") for m in range(2)]
            for n in range(4):
                for m in range(2):
                    fp = fpp.tile([128, 512], F32, tag=f"f{m}", name=f"f{m}")
                    for e in range(4):
                        nc.tensor.matmul(fp[:],
                                         e1b[e][:, 128 * m:128 * m + 128],
                                         gsb[e][:, 512 * n:512 * n + 512],
                                         start=(e == 0), stop=(e == 3))
                    nc.vector.tensor_scalar_mul(
                        osb[m][:, 512 * n:512 * n + 512], fp[:], rnt[m][:])
            for m in range(2):
                nc.sync.dma_start(out_d[128 * m:128 * m + 128, :], osb[m][:])

    nc.compile()
    return nc


def make_in_maps(shared, percore):
    import ml_dtypes
    bf = ml_dtypes.bfloat16

    def b16(x):
        return np.asarray(x, np.float32).astype(bf)

    base = {
        "op_embt1": b16(shared["op_emb"]),
        "op_embt2": np.ascontiguousarray(shared["op_emb"], np.float32),
        "op_c1t1": b16(shared["op_c1t1"]),
        "op_c1t2": np.ascontiguousarray(shared["op_c1t2"], np.float32),
        "op_c2t1": b16(shared["op_c2t1"]),
        "op_c2t2": np.ascontiguousarray(shared["op_c2t2"], np.float32),
        "lw1": b16(shared["lw_t1"]),
        "lw2": np.ascontiguousarray(shared["lw_t2"], np.float32),
        "b_se": shared["b_emb"], "b_c1t1": shared["b_c1t1"],
        "b_c1t2": shared["b_c1t2"], "b_c2t1": shared["b_c2t1"],
        "b_c2t2": shared["b_c2t2"],
        "b_l1": shared["b_l1"], "b_l2": shared["b_l2"],
        "znt": shared["znT"], "zn": shared["zn"],
        "ident": np.eye(128, dtype=np.float32),
    }
    maps = []
    for pc in percore:
        m = dict(base)
        m["ohs"] = np.ascontiguousarray(pc["ohs"])
        m["ohd"] = np.ascontiguousarray(pc["ohd"])
        maps.append(m)
    return maps


def kernel(**inputs):
    dsf = np.asarray(inputs.get("downscale_factor", 1)).reshape(-1)
    dsf = int(dsf[0]) if dsf.size else 1
    assert dsf == 1, f"only downscale_factor=1 supported, got {dsf}"
    shared, percore, esc = host_prep(inputs)
    nc = build_program(esc, debug=DEBUG)
    maps = make_in_maps(shared, percore)
    res = run_bass_kernel_spmd(nc, maps, list(range(NCORES)))
    out = np.concatenate([res.results[c]["out"] for c in range(NCORES)],
                         axis=0)
    return out.astype(np.float32)


KERNEL_RESULTS = {}


def run_for_test(inputs, trace=False):
    """test.py hook: returns (out, per-core results, BassKernelResults)."""
    shared, percore, esc = host_prep(inputs)
    nc = build_program(esc, debug=DEBUG)
    maps = make_in_maps(shared, percore)
    res = run_bass_kernel_spmd(nc, maps, list(range(NCORES)), trace=trace)
    out = np.concatenate([res.results[c]["out"] for c in range(NCORES)],
                         axis=0)
    return out.astype(np.float32), res

